# revision 1
# baseline (speedup 1.0000x reference)
"""Trainium2 Bass kernel for nn_BRC_17179869451 (BRC-style RNN).

  xz/xr/xh = x @ {kz,kr,kh}   (three [B*T,D]x[D,H] GEMMs)
  scan over T:
      r = tanh(xr_t + h*mr + br) + 1
      z = sigmoid(xz_t + h*mz + bz)
      h = z*h + (1-z)*tanh(xh_t + r*h)

Sharding: batch dim (B=64) split across 8 cores (8 batches each); weights
replicated; the sequential scan runs locally per shard.

Device-side formulation (shifted state hh = h + 1, so every +1 constant
folds into GEMM epilogue biases / fused scalar_tensor_tensor ops):
  XZ = xz + bz - mz            (epilogue: scale 1, bias bz-mz)
  XR = 2*(xr + br - mr)        (epilogue: scale 2, bias 2*(br-mr))
  XH = 2*xh                    (epilogue: scale 2)
  per step:
    e2 = 2*hh + XR      ; s = sigmoid(e2)        # r+1 = 2s  (fast path mr=1)
    e1 = hh + XZ        ; z = sigmoid(e1)        # (fast path mz=1)
    p  = (hh-1)*s
    e3 = 4*p + XH       ; q = sigmoid(e3)        # tanh(xh+2sh) = 2q-1
    dh = hh - 2*q
    w  = dh*z
    hh' = w + 2*q                                 # = h' + 1
Layout per core: state [128 x 64]: partition = h mod 128 (h_a),
free = (h_b = h div 128 [8], b [8]).  GEMM: out[h_a, (t,b)] =
kz[:, 128*h_b:128*(h_b+1)]^T @ x^T, x transposed on-chip via PE.
Output: PE re-transpose of the state ring -> [ (t2,h_b,b), h_a ] so the
DMA to ys[b,t,h] writes 512B-contiguous runs; the -1 un-shift folds into
the post-transpose copy bias.
"""

import os
import numpy as np

B, T, D, H = 64, 512, 512, 1024
NCORES = 8
BL = B // NCORES          # 8 batches per core
TC = 32                   # timesteps per chunk
NCH = T // TC             # 16 chunks
HB = H // 128             # 8 h-blocks
FS = HB * BL              # 64 = free size of scan state
KT = D // 128             # 4 k-tiles

_cache = {}


def _apply_tile_drain_patch():
    """Spread end-of-kernel sem waits over single-wait sync nops: walrus
    CoreV3 codegen rejects the stock Tile exit Drain that carries one wait
    per logical proc ("Too many sync wait commands")."""
    import concourse.tile as tile_mod

    if getattr(tile_mod.TileContext, "_drain_patched", False):
        return

    def _patched(self, tick_clock, wait_clock):
        from concourse.vector_clock import ScopedClock

        vclock = tick_clock.global_clock
        pend = [(p, vclock[p]) for p in range(len(vclock)) if vclock[p] > 0]
        for proc, tick in pend:
            sub = ScopedClock()
            sub.require_at_least(None, proc, tick)
            nop_inst = self.nc.sync.nop(nofuse=True)
            wait_clock.add_sem_waits(nop_inst.ins, sub)
        self.nc.sync.drain()
        self.nc.all_engine_barrier()
        assert self.sems is not None
        popped = self.nc._tile_sem_poison_stack.pop()
        assert popped is self._sem_poison
        self.nc.clear_and_free_semaphores(list(self.sems.allocated().values()))
        self.nc.all_engine_barrier()

    tile_mod.TileContext._drain_and_barrier = _patched
    tile_mod.TileContext._drain_patched = True


def _legalize_sync_waits(nc, max_waits: int = 1):
    """walrus codegen here rejects instructions with >1 sem wait ("Too many
    sync wait commands"); hoist extra waits onto same-engine NoOps."""
    import concourse.mybir as mybir

    n = 0
    for f in nc.m.functions:
        for bb in f.blocks:
            out = []
            for ins in bb.instructions:
                si = ins.sync_info
                if si is not None and si.on_wait and len(si.on_wait) > max_waits:
                    waits = list(si.on_wait)
                    for w in waits[:-max_waits]:
                        n += 1
                        nop = mybir.InstNoOp(
                            name=f"waitnop_{n}", engine=ins.engine)
                        nop.sync_info = mybir.SyncInfo(
                            on_wait=[w], on_update=[])
                        out.append(nop)
                    si.on_wait = waits[-max_waits:]
                out.append(ins)
            bb.instructions = out


def _build(fast: bool):
    import concourse.bass as bass
    import concourse.mybir as mybir
    from concourse.tile import TileContext
    from concourse.masks import make_identity

    _apply_tile_drain_patch()

    fp32 = mybir.dt.float32
    AF = mybir.ActivationFunctionType
    OP = mybir.AluOpType

    nc = bass.Bass()
    x_d = nc.dram_tensor("x", [BL, T, D], fp32, kind="ExternalInput")
    kz_d = nc.dram_tensor("kz", [D, H], fp32, kind="ExternalInput")
    kr_d = nc.dram_tensor("kr", [D, H], fp32, kind="ExternalInput")
    kh_d = nc.dram_tensor("kh", [D, H], fp32, kind="ExternalInput")
    # epilogue bias vectors, host-precomputed, [128, HB] (p=h_a, f=h_b)
    bzv_d = nc.dram_tensor("bzv", [128, HB], fp32, kind="ExternalInput")
    brv_d = nc.dram_tensor("brv", [128, HB], fp32, kind="ExternalInput")
    if not fast:
        mzt_d = nc.dram_tensor("mzt", [128, FS], fp32, kind="ExternalInput")
        mr2t_d = nc.dram_tensor("mr2t", [128, FS], fp32, kind="ExternalInput")
    ys_d = nc.dram_tensor("ys", [BL, T, H], fp32, kind="ExternalOutput")

    with TileContext(nc) as tc:
        with (
            tc.tile_pool(name="const", bufs=1) as cpool,
            tc.tile_pool(name="xraw", bufs=3) as xraw_pool,
            tc.tile_pool(name="xT", bufs=2) as xT_pool,
            tc.tile_pool(name="gates", bufs=3) as gate_pool,
            tc.tile_pool(name="ring", bufs=3) as ring_pool,
            tc.tile_pool(name="stg", bufs=3) as stg_pool,
            tc.tile_pool(name="scan", bufs=3) as scan_pool,
            tc.tile_pool(name="psmm", bufs=3, space="PSUM") as psmm_pool,
            tc.tile_pool(name="pstp", bufs=2, space="PSUM") as pstp_pool,
            tc.tile_pool(name="psyt", bufs=2, space="PSUM") as psyt_pool,
        ):
            # ---- constants / weights ----
            ident = cpool.tile([128, 128], fp32, tag="ident")
            make_identity(nc, ident)

            w_sb = {}
            for name, wd in (("z", kz_d), ("r", kr_d), ("h", kh_d)):
                for k in range(KT):
                    wt = cpool.tile([128, H], fp32, tag=f"w{name}{k}")
                    nc.sync.dma_start(out=wt, in_=wd[k * 128:(k + 1) * 128, :])
                    w_sb[(name, k)] = wt
            bzv = cpool.tile([128, HB], fp32, tag="bzv")
            nc.sync.dma_start(out=bzv, in_=bzv_d[:, :])
            brv = cpool.tile([128, HB], fp32, tag="brv")
            nc.sync.dma_start(out=brv, in_=brv_d[:, :])
            if not fast:
                mzt = cpool.tile([128, FS], fp32, tag="mzt")
                nc.sync.dma_start(out=mzt, in_=mzt_d[:, :])
                mr2t = cpool.tile([128, FS], fp32, tag="mr2t")
                nc.sync.dma_start(out=mr2t, in_=mr2t_d[:, :])

            h_init = cpool.tile([128, FS], fp32, tag="hinit")
            nc.vector.memset(h_init, 1.0)  # hh0 = h0 + 1 = 1
            negone = cpool.tile([128, 1], fp32, tag="negone")
            nc.vector.memset(negone, -1.0)

            prev_state = h_init  # AP of previous step's state tile

            import bass_rust as _br

            chunk_gates = {}
            _pe_last = [None]
            _act_last = [None]

            def act_dep(bi):
                if _act_last[0] is not None:
                    _br.add_dep_helper(bi.ins, _act_last[0].ins, sync=False,
                                       reason="act emission order")
                _act_last[0] = bi
                return bi

            def pe_dep(bi):
                # Pin PE stream to emission order (in-order engine anyway);
                # prevents scheduler priority inversions that serialize the
                # chunk pipeline.
                if _pe_last[0] is not None:
                    _br.add_dep_helper(bi.ins, _pe_last[0].ins, sync=False,
                                       reason="pe emission order")
                _pe_last[0] = bi

            def make_gemm_pieces(c):
                """Closures emitting chunk c's GEMM work, one piece per
                scan step of the previous chunk (software pipelining by
                emission order)."""
                t0 = c * TC
                xT = [xT_pool.tile([128, TC * BL], fp32, tag=f"xT{k}",
                                   name=f"xT{k}_{c}") for k in range(KT)]
                XZ = gate_pool.tile([128, TC * FS], fp32, tag="XZ",
                                    name=f"XZ_{c}")
                XR = gate_pool.tile([128, TC * FS], fp32, tag="XR",
                                    name=f"XR_{c}")
                XH = gate_pool.tile([128, TC * FS], fp32, tag="XH",
                                    name=f"XH_{c}")
                chunk_gates[c] = (XZ, XR, XH)
                pieces = []
                for s in range(TC // 16):
                    xrow = xraw_pool.tile([128, D], fp32, tag="xraw",
                                          name=f"xrow_{c}_{s}")

                    def load(s=s, xrow=xrow):
                        # SWDGE (gpsimd) path: keeps the input stream's DMA
                        # queue rotation decoupled from the scan-gated ys
                        # output DMAs on the SP HWDGE queues.
                        nc.gpsimd.dma_start(
                            out=xrow,
                            in_=x_d[:, t0 + s * 16: t0 + (s + 1) * 16, :])
                    pieces.append(load)

                    def tr(s=s, xrow=xrow, c=c):
                        for k in range(KT):
                            tp = pstp_pool.tile([128, 128], fp32, tag="tp",
                                                name=f"tp_{c}_{s}_{k}")
                            pe_dep(nc.tensor.transpose(
                                tp, xrow[:, k * 128:(k + 1) * 128], ident))
                            nc.vector.tensor_copy(
                                xT[k][:, s * 128:(s + 1) * 128], tp)
                    pieces.append(tr)
                for hb in range(HB):
                    for gname, dest, scale, bias in (
                        ("z", XZ, 1.0, bzv[:, hb:hb + 1]),
                        ("r", XR, 2.0, brv[:, hb:hb + 1]),
                        ("h", XH, 2.0, 0.0),
                    ):
                        def mmgroup(gname=gname, dest=dest, scale=scale,
                                    bias=bias, hb=hb, c=c):
                            ps = psmm_pool.tile([128, TC * BL], fp32,
                                                tag="mm",
                                                name=f"mm_{c}_{gname}_{hb}")
                            for k in range(KT):
                                pe_dep(nc.tensor.matmul(
                                    out=ps,
                                    lhsT=w_sb[(gname, k)][
                                        :, hb * 128:(hb + 1) * 128],
                                    rhs=xT[k],
                                    start=(k == 0), stop=(k == KT - 1)))
                            dst4 = dest.rearrange(
                                "p (s t r) -> p s t r", s=TC // 16, t=16)[
                                :, :, :, hb * BL:(hb + 1) * BL]
                            ps4 = ps.rearrange(
                                "p (s b t) -> p s t b", s=TC // 16, b=BL)
                            act_dep(nc.scalar.activation(
                                out=dst4, in_=ps4,
                                func=AF.Identity, bias=bias, scale=scale))
                        pieces.append(mmgroup)
                return pieces

            def emit_out_piece(out_info, j):
                osc, oring, ostg, ot0 = out_info
                yt = psyt_pool.tile([128, 128], fp32, tag="ytp",
                                    name=f"yt_{osc}_{j}")
                pe_dep(nc.tensor.transpose(
                    yt, oring[:, j * 128:(j + 1) * 128], ident))
                nc.vector.tensor_scalar(
                    out=ostg[:, j * 128:(j + 1) * 128], in0=yt,
                    scalar1=-1.0, scalar2=None, op0=OP.add)
                dst = ys_d[:, ot0 + 2 * j:ot0 + 2 * j + 2, :].rearrange(
                    "b t (hb ha) -> t hb b ha", ha=128)
                nc.sync.dma_start(
                    out=dst, in_=ostg[:, j * 128:(j + 1) * 128])

            prev_out = None

            for p in make_gemm_pieces(0):
                p()
            for p in make_gemm_pieces(1):
                p()

            for sc in range(NCH):
                ring = ring_pool.tile([128, TC * FS], fp32, tag="ring",
                                      name=f"ring_{sc}")
                stg = stg_pool.tile([128, TC * FS], fp32, tag="stg",
                                    name=f"stg_{sc}")
                nxt = make_gemm_pieces(sc + 2) if sc + 2 < NCH else []
                XZ, XR, XH = chunk_gates[sc]
                t0 = sc * TC
                pi = 0
                for t in range(TC):
                    fs = slice(t * FS, (t + 1) * FS)
                    hh = prev_state
                    xz_t, xr_t_, xh_t = XZ[:, fs], XR[:, fs], XH[:, fs]
                    # chain: e2 -> s -> p -> e3 -> q -> v -> ring
                    # off-chain: e1 -> z -> u=1-z, zh=z*hh
                    # ring = 2q(1-z) + z*hh  ==  z(hh-2q) + 2q
                    e2 = scan_pool.tile([128, FS], fp32, tag="e2",
                                        name=f"e2_{sc}_{t}")
                    e1 = scan_pool.tile([128, FS], fp32, tag="e1",
                                        name=f"e1_{sc}_{t}")
                    if fast:
                        nc.vector.scalar_tensor_tensor(
                            out=e2, in0=hh, scalar=2.0, in1=xr_t_,
                            op0=OP.mult, op1=OP.add)
                        nc.gpsimd.tensor_tensor(e1, hh, xz_t, OP.add)
                    else:
                        m2 = scan_pool.tile([128, FS], fp32, tag="m2",
                                            name=f"m2_{sc}_{t}")
                        nc.vector.tensor_tensor(m2, hh, mr2t, OP.mult)
                        nc.vector.tensor_tensor(e2, m2, xr_t_, OP.add)
                        m1 = scan_pool.tile([128, FS], fp32, tag="m1",
                                            name=f"m1_{sc}_{t}")
                        nc.gpsimd.tensor_tensor(m1, hh, mzt, OP.mult)
                        nc.gpsimd.tensor_tensor(e1, m1, xz_t, OP.add)
                    s_t = scan_pool.tile([128, FS], fp32, tag="s",
                                         name=f"s_{sc}_{t}")
                    s_i = act_dep(nc.scalar.activation(s_t, e2, AF.Sigmoid))
                    p_t = scan_pool.tile([128, FS], fp32, tag="p",
                                         name=f"p_{sc}_{t}")
                    nc.vector.scalar_tensor_tensor(
                        out=p_t, in0=hh, scalar=1.0, in1=s_t,
                        op0=OP.subtract, op1=OP.mult)
                    e3 = scan_pool.tile([128, FS], fp32, tag="e3",
                                        name=f"e3_{sc}_{t}")
                    nc.vector.scalar_tensor_tensor(
                        out=e3, in0=p_t, scalar=4.0, in1=xh_t,
                        op0=OP.mult, op1=OP.add)
                    q_t = scan_pool.tile([128, FS], fp32, tag="q",
                                         name=f"q_{sc}_{t}")
                    q_i = act_dep(nc.scalar.activation(q_t, e3, AF.Sigmoid))
                    z_t = scan_pool.tile([128, FS], fp32, tag="z",
                                         name=f"z_{sc}_{t}")
                    z_i = act_dep(nc.scalar.activation(z_t, e1, AF.Sigmoid))
                    u_t = scan_pool.tile([128, FS], fp32, tag="u",
                                         name=f"u_{sc}_{t}")
                    nc.vector.tensor_scalar(
                        out=u_t, in0=z_t, scalar1=-1.0, scalar2=1.0,
                        op0=OP.mult, op1=OP.add)
                    zh = scan_pool.tile([128, FS], fp32, tag="zh",
                                        name=f"zh_{sc}_{t}")
                    nc.gpsimd.tensor_tensor(zh, z_t, hh, OP.mult)
                    v_t = scan_pool.tile([128, FS], fp32, tag="v",
                                         name=f"v_{sc}_{t}")
                    nc.vector.scalar_tensor_tensor(
                        out=v_t, in0=q_t, scalar=2.0, in1=u_t,
                        op0=OP.mult, op1=OP.mult)
                    nc.vector.tensor_tensor(ring[:, fs], v_t, zh, OP.add)
                    prev_state = ring[:, fs]

                    if pi < len(nxt):
                        nxt[pi]()
                        pi += 1
                    if t % 2 == 1 and prev_out is not None:
                        emit_out_piece(prev_out, (t - 1) // 2)
                while pi < len(nxt):
                    nxt[pi]()
                    pi += 1
                prev_out = (sc, ring, stg, t0)

            # flush the last chunk's output
            for j in range(TC * FS // 128):
                emit_out_piece(prev_out, j)

    _legalize_sync_waits(nc)
    return nc


def _get_nc(fast: bool):
    if fast not in _cache:
        _cache[fast] = _build(fast)
    return _cache[fast]


LAST_RESULT = None


def kernel(**inputs):
    global LAST_RESULT
    from concourse.bass_utils import run_bass_kernel_spmd

    x = np.ascontiguousarray(np.asarray(inputs["x"], dtype=np.float32))
    kz = np.ascontiguousarray(np.asarray(inputs["kz"], dtype=np.float32))
    kr = np.ascontiguousarray(np.asarray(inputs["kr"], dtype=np.float32))
    kh = np.ascontiguousarray(np.asarray(inputs["kh"], dtype=np.float32))
    mz = np.asarray(inputs["mz"], dtype=np.float32)
    mr = np.asarray(inputs["mr"], dtype=np.float32)
    br = np.asarray(inputs["br"], dtype=np.float32)
    bz = np.asarray(inputs["bz"], dtype=np.float32)
    assert x.shape == (B, T, D) and kz.shape == (D, H)

    fast = bool(np.all(mz == 1.0) and np.all(mr == 1.0))
    nc = _get_nc(fast)

    # [H] -> [128, HB] with [h_a, h_b] = v[h_b*128 + h_a]
    def pvec(v):
        return np.ascontiguousarray(v.reshape(HB, 128).T)

    bzv = pvec(bz - mz)
    brv = pvec(2.0 * (br - mr))
    base = {"kz": kz, "kr": kr, "kh": kh, "bzv": bzv, "brv": brv}
    if not fast:
        # [128, (hb, b)] tiles of mz / 2*mr broadcast over b
        def ptile(v):
            t = v.reshape(HB, 128).T  # [128, HB]
            return np.ascontiguousarray(
                np.repeat(t[:, :, None], BL, axis=2).reshape(128, FS))
        base["mzt"] = ptile(mz)
        base["mr2t"] = ptile(2.0 * mr)

    in_maps = [dict(base, x=np.ascontiguousarray(x[i * BL:(i + 1) * BL]))
               for i in range(NCORES)]

    trace = bool(int(os.environ.get("KERNEL_TRACE", "0")))
    res = run_bass_kernel_spmd(nc, in_maps, list(range(NCORES)), trace=trace)
    LAST_RESULT = res
    ys = np.concatenate([res.results[i]["ys"] for i in range(NCORES)], axis=0)
    return ys



# revision 3
# speedup vs baseline: 1.4222x; 1.4222x over previous
"""Trainium2 Bass kernel for nn_BRC_17179869451 (BRC-style RNN).

  xz/xr/xh = x @ {kz,kr,kh}   (three [B*T,D]x[D,H] GEMMs)
  scan over T:
      r = tanh(xr_t + h*mr + br) + 1
      z = sigmoid(xz_t + h*mz + bz)
      h = z*h + (1-z)*tanh(xh_t + r*h)

Sharding: batch dim (B=64) split across 8 cores (8 batches each); weights
replicated; the sequential scan runs locally per shard.

v2 design:
- x is pre-transposed AND cast to bf16 on the HOST into [128, (c,k,s,b,t)]
  layout, so the GEMM rhs tiles DMA straight into SBUF: no on-chip input
  transposes.  Weights are bf16 too: PE matmuls run at 1 col/cycle instead
  of fp32's 1/4 rate.  PSUM accumulation stays fp32.
- Scan (fast path mz=mr=1), shifted state hh = h + 1:
    XZ  = xz + bz - 1        (epilogue scale 1, bias bz-mz)
    XR2 = xr + br - 1        (epilogue scale 1, bias br-mr)
    XH2 = xh / 2             (epilogue scale 0.5)
  per step (engine in brackets; sigma-scale folded into ACT):
    e2h = v' + cH'           [V]   (= hh + XR2; cH' = zh' + XR2_t)
    hh  = v' + zh'           [V]   (ring slab write, also the output)
    e1  = v' + czh'          [G]   (= hh + XZ; czh' = zh' + XZ_t)
    s   = sigmoid(2*e2h)     [S]
    z   = sigmoid(e1)        [S]
    p   = (hh-1)*s           [V]
    e3h = p + XH2_t          [V]
    zh  = z*hh               [V]
    u   = 1-z                [G]
    q   = sigmoid(4*e3h)     [S]
    cH  = zh + XR2_{t+1}     [G]
    czh = zh + XZ_{t+1}      [G]
    v   = 2q*u               [V]
  The e2h/e1 "from v" factoring keeps the ring add OFF the critical cycle;
  z is scheduled before q's consumer so u is ready early.
- Output: hh slab -> PE transpose -> V copy -> DMA; the -1 un-shift is done
  on the HOST after gather (ys = dev_out - 1).
"""

import os
import numpy as np

B, T, D, H = 64, 512, 512, 1024
NCORES = 8
BL = B // NCORES          # 8 batches per core
TC = 32                   # timesteps per chunk
NCH = T // TC             # 16 chunks
HB = H // 128             # 8 h-blocks
FS = HB * BL              # 64 = free size of scan state
KT = D // 128             # 4 k-tiles

_cache = {}


def _apply_tile_drain_patch():
    """Spread end-of-kernel sem waits over single-wait sync nops: walrus
    CoreV3 codegen rejects the stock Tile exit Drain that carries one wait
    per logical proc ("Too many sync wait commands")."""
    import concourse.tile as tile_mod

    if getattr(tile_mod.TileContext, "_drain_patched", False):
        return

    def _patched(self, tick_clock, wait_clock):
        from concourse.vector_clock import ScopedClock

        vclock = tick_clock.global_clock
        pend = [(p, vclock[p]) for p in range(len(vclock)) if vclock[p] > 0]
        for proc, tick in pend:
            sub = ScopedClock()
            sub.require_at_least(None, proc, tick)
            nop_inst = self.nc.sync.nop(nofuse=True)
            wait_clock.add_sem_waits(nop_inst.ins, sub)
        self.nc.sync.drain()
        self.nc.all_engine_barrier()
        assert self.sems is not None
        popped = self.nc._tile_sem_poison_stack.pop()
        assert popped is self._sem_poison
        self.nc.clear_and_free_semaphores(list(self.sems.allocated().values()))
        self.nc.all_engine_barrier()

    tile_mod.TileContext._drain_and_barrier = _patched
    tile_mod.TileContext._drain_patched = True


def _legalize_sync_waits(nc, max_waits: int = 1):
    """walrus codegen here rejects instructions with >1 sem wait ("Too many
    sync wait commands"); hoist extra waits onto same-engine NoOps."""
    import concourse.mybir as mybir

    n = 0
    for f in nc.m.functions:
        for bb in f.blocks:
            out = []
            for ins in bb.instructions:
                si = ins.sync_info
                if si is not None and si.on_wait and len(si.on_wait) > max_waits:
                    waits = list(si.on_wait)
                    for w in waits[:-max_waits]:
                        n += 1
                        nop = mybir.InstNoOp(
                            name=f"waitnop_{n}", engine=ins.engine)
                        nop.sync_info = mybir.SyncInfo(
                            on_wait=[w], on_update=[])
                        out.append(nop)
                    si.on_wait = waits[-max_waits:]
                out.append(ins)
            bb.instructions = out


def _build(fast: bool):
    import concourse.bass as bass
    import concourse.mybir as mybir
    from concourse.tile import TileContext
    from concourse.masks import make_identity

    _apply_tile_drain_patch()

    fp32 = mybir.dt.float32
    bf16 = mybir.dt.bfloat16
    AF = mybir.ActivationFunctionType
    OP = mybir.AluOpType

    nc = bass.Bass()
    XT_F = NCH * KT * TC * BL  # 16384; layout (c, k, s, b, t)
    xt_d = nc.dram_tensor("xt", [128, XT_F], bf16, kind="ExternalInput")
    kz_d = nc.dram_tensor("kz", [D, H], bf16, kind="ExternalInput")
    kr_d = nc.dram_tensor("kr", [D, H], bf16, kind="ExternalInput")
    kh_d = nc.dram_tensor("kh", [D, H], bf16, kind="ExternalInput")
    # epilogue bias vectors, host-precomputed, [128, HB] (p=h_a, f=h_b)
    bzv_d = nc.dram_tensor("bzv", [128, HB], fp32, kind="ExternalInput")
    brv_d = nc.dram_tensor("brv", [128, HB], fp32, kind="ExternalInput")
    if not fast:
        mzt_d = nc.dram_tensor("mzt", [128, FS], fp32, kind="ExternalInput")
        mr2t_d = nc.dram_tensor("mr2t", [128, FS], fp32, kind="ExternalInput")
    ys_d = nc.dram_tensor("ys", [BL, T, H], fp32, kind="ExternalOutput")

    with TileContext(nc) as tc:
        with (
            tc.tile_pool(name="const", bufs=1) as cpool,
            tc.tile_pool(name="xt", bufs=2) as xt_pool,
            tc.tile_pool(name="gates", bufs=3) as gate_pool,
            tc.tile_pool(name="ring", bufs=3) as ring_pool,
            tc.tile_pool(name="stg", bufs=4) as stg_pool,
            tc.tile_pool(name="scan", bufs=3) as scan_pool,
            tc.tile_pool(name="psmm", bufs=4, space="PSUM") as psmm_pool,
            tc.tile_pool(name="psyt", bufs=2, space="PSUM") as psyt_pool,
        ):
            # ---- constants / weights ----
            ident = cpool.tile([128, 128], fp32, tag="ident")
            make_identity(nc, ident)

            w_sb = {}
            for name, wd in (("z", kz_d), ("r", kr_d), ("h", kh_d)):
                for k in range(KT):
                    wt = cpool.tile([128, H], bf16, tag=f"w{name}{k}")
                    nc.sync.dma_start(out=wt, in_=wd[k * 128:(k + 1) * 128, :])
                    w_sb[(name, k)] = wt
            bzv = cpool.tile([128, HB], fp32, tag="bzv")
            nc.sync.dma_start(out=bzv, in_=bzv_d[:, :])
            brv = cpool.tile([128, HB], fp32, tag="brv")
            nc.sync.dma_start(out=brv, in_=brv_d[:, :])
            if not fast:
                mzt = cpool.tile([128, FS], fp32, tag="mzt")
                nc.sync.dma_start(out=mzt, in_=mzt_d[:, :])
                mr2t = cpool.tile([128, FS], fp32, tag="mr2t")
                nc.sync.dma_start(out=mr2t, in_=mr2t_d[:, :])

            h_init = cpool.tile([128, FS], fp32, tag="hinit")
            nc.vector.memset(h_init, 1.0)  # hh0 = h0 + 1 = 1
            zeros0 = cpool.tile([128, FS], fp32, tag="zeros0")
            nc.vector.memset(zeros0, 0.0)

            import bass_rust as _br

            _last = {"pe": None, "act": None, "vec": None, "gps": None}

            def _dep(key, bi):
                if _last[key] is not None:
                    _br.add_dep_helper(bi.ins, _last[key].ins, sync=False,
                                       reason=f"{key} emission order")
                _last[key] = bi
                return bi

            def pe_dep(bi):
                return _dep("pe", bi)

            def act_dep(bi):
                return _dep("act", bi)

            def vec_dep(bi):
                return _dep("vec", bi)

            def gps_dep(bi):
                return _dep("gps", bi)

            chunk_gates = {}

            # epilogue (scale, bias) per gate
            if fast:
                epi = {"z": (1.0, "bzv"), "r": (1.0, "brv"), "h": (0.5, None)}
            else:
                epi = {"z": (1.0, "bzv"), "r": (2.0, "brv"), "h": (2.0, None)}

            def make_gemm_pieces(c):
                """Closures emitting chunk c's GEMM work, one piece per
                scan step (software pipelining by emission order)."""
                xt_t = xt_pool.tile([128, KT * TC * BL], bf16, tag="xt",
                                    name=f"xt_{c}")
                XZ = gate_pool.tile([128, TC * FS], fp32, tag="XZ",
                                    name=f"XZ_{c}")
                XR = gate_pool.tile([128, TC * FS], fp32, tag="XR",
                                    name=f"XR_{c}")
                XH = gate_pool.tile([128, TC * FS], fp32, tag="XH",
                                    name=f"XH_{c}")
                chunk_gates[c] = (XZ, XR, XH)
                pieces = []

                def load(c=c, xt_t=xt_t):
                    nc.sync.dma_start(
                        out=xt_t,
                        in_=xt_d[:, c * KT * TC * BL:(c + 1) * KT * TC * BL])
                pieces.append(load)
                for hb in range(HB):
                    for gname, dest in (("z", XZ), ("r", XR), ("h", XH)):
                        def mmgroup(gname=gname, dest=dest, hb=hb, c=c,
                                    xt_t=xt_t):
                            scale, bname = epi[gname]
                            bias = 0.0
                            if bname == "bzv":
                                bias = bzv[:, hb:hb + 1]
                            elif bname == "brv":
                                bias = brv[:, hb:hb + 1]
                            ps = psmm_pool.tile([128, TC * BL], fp32,
                                                tag="mm",
                                                name=f"mm_{c}_{gname}_{hb}")
                            for k in range(KT):
                                pe_dep(nc.tensor.matmul(
                                    out=ps,
                                    lhsT=w_sb[(gname, k)][
                                        :, hb * 128:(hb + 1) * 128],
                                    rhs=xt_t[:, k * TC * BL:(k + 1) * TC * BL],
                                    start=(k == 0), stop=(k == KT - 1)))
                            dst4 = dest.rearrange(
                                "p (s t r) -> p s t r", s=TC // 16, t=16)[
                                :, :, :, hb * BL:(hb + 1) * BL]
                            ps4 = ps.rearrange(
                                "p (s b t) -> p s t b", s=TC // 16, b=BL)
                            act_dep(nc.scalar.activation(
                                out=dst4, in_=ps4,
                                func=AF.Identity, bias=bias, scale=scale))
                        pieces.append(mmgroup)
                return pieces

            # outstanding output work for the previous chunk
            # (ring slab AP, t0); transposes emitted on even steps, the
            # psum->sbuf copy + DMA on the following odd step.
            def emit_out_transpose(oring, j, sc):
                yt = psyt_pool.tile([128, 128], fp32, tag="ytp",
                                    name=f"yt_{sc}_{j}")
                pe_dep(nc.tensor.transpose(
                    yt, oring[:, j * 128:(j + 1) * 128], ident))
                return yt

            def emit_out_store(yt, ot0, j, sc):
                stg = stg_pool.tile([128, 128], fp32, tag="stg",
                                    name=f"stg_{sc}_{j}")
                vec_dep(nc.vector.tensor_copy(stg, yt))
                dst = ys_d[:, ot0 + 2 * j:ot0 + 2 * j + 2, :].rearrange(
                    "b t (hb ha) -> t hb b ha", ha=128)
                nc.sync.dma_start(out=dst, in_=stg)

            for p in make_gemm_pieces(0):
                p()
            for p in make_gemm_pieces(1):
                p()

            if fast:
                # prev-step tiles: (v, zh, cH, czh); virtual step -1:
                # hh_0 = 1 -> v=ones, zh=zeros, cH = XR2_0, czh = XZ_0
                XZ0, XR0, _ = chunk_gates[0]
                prev = (h_init, zeros0, XR0[:, 0:FS], XZ0[:, 0:FS])
                prev_hh = h_init
                prev_out = None     # (ring slab, t0, sc)
                pend_yt = None      # psum transpose tile awaiting store

                for sc in range(NCH):
                    ring = ring_pool.tile([128, TC * FS], fp32, tag="ring",
                                          name=f"ring_{sc}")
                    nxt = make_gemm_pieces(sc + 2) if sc + 2 < NCH else []
                    XZ, XR, XH = chunk_gates[sc]
                    t0 = sc * TC
                    pi = 0
                    for t in range(TC):
                        g = t0 + t
                        fs = slice(t * FS, (t + 1) * FS)
                        v_p, zh_p, cH_p, czh_p = prev

                        def stile(tag):
                            return scan_pool.tile([128, FS], fp32, tag=tag,
                                                  name=f"{tag}_{sc}_{t}")

                        # 1. e2h = v' + cH'
                        e2h = stile("e2h")
                        vec_dep(nc.vector.tensor_tensor(
                            e2h, v_p, cH_p, OP.add))
                        # 2. ring write hh_g into slab slot g-1
                        if g > 0:
                            o = (g - 1) % TC
                            oslab = ring if t > 0 else prev_ring
                            hh = oslab[:, o * FS:(o + 1) * FS]
                            vec_dep(nc.vector.tensor_tensor(
                                hh, v_p, zh_p, OP.add))
                        else:
                            hh = h_init
                        # 2.5 output store for pending transpose (odd steps)
                        if t % 2 == 1 and pend_yt is not None:
                            oring, ot0, osc, j = pend_yt
                            emit_out_store(oring, ot0, j, osc)
                            pend_yt = None
                        # 3. e1 = v' + czh'
                        e1 = stile("e1")
                        gps_dep(nc.gpsimd.tensor_tensor(
                            e1, v_p, czh_p, OP.add))
                        # 4. s = sigmoid(2*e2h)
                        s_t = stile("s")
                        act_dep(nc.scalar.activation(
                            s_t, e2h, AF.Sigmoid, scale=2.0))
                        # 5. z = sigmoid(e1)
                        z_t = stile("z")
                        act_dep(nc.scalar.activation(z_t, e1, AF.Sigmoid))
                        # 6. p = (hh-1)*s
                        p_t = stile("p")
                        vec_dep(nc.vector.scalar_tensor_tensor(
                            out=p_t, in0=hh, scalar=1.0, in1=s_t,
                            op0=OP.subtract, op1=OP.mult))
                        # 7. e3h = p + XH2
                        e3h = stile("e3h")
                        vec_dep(nc.vector.tensor_tensor(
                            e3h, p_t, XH[:, fs], OP.add))
                        # 8. zh = z*hh
                        zh_t = stile("zh")
                        vec_dep(nc.vector.tensor_tensor(
                            zh_t, z_t, hh, OP.mult))
                        # 9. u = 1-z
                        u_t = stile("u")
                        gps_dep(nc.gpsimd.tensor_scalar(
                            out=u_t, in0=z_t, scalar1=-1.0, scalar2=1.0,
                            op0=OP.mult, op1=OP.add))
                        # 10. q = sigmoid(4*e3h)
                        q_t = stile("q")
                        act_dep(nc.scalar.activation(
                            q_t, e3h, AF.Sigmoid, scale=4.0))
                        # 11/12. cH = zh + XR2_{t+1}; czh = zh + XZ_{t+1}
                        if g + 1 < T:
                            if t + 1 < TC:
                                XZn, XRn = XZ, XR
                                nfs = slice((t + 1) * FS, (t + 2) * FS)
                            else:
                                XZn, XRn, _ = chunk_gates[sc + 1]
                                nfs = slice(0, FS)
                            cH_t = stile("cH")
                            gps_dep(nc.gpsimd.tensor_tensor(
                                cH_t, zh_t, XRn[:, nfs], OP.add))
                            czh_t = stile("czh")
                            gps_dep(nc.gpsimd.tensor_tensor(
                                czh_t, zh_t, XZn[:, nfs], OP.add))
                        else:
                            cH_t, czh_t = None, None
                        # 13. v = 2q*u
                        v_t = stile("v")
                        vec_dep(nc.vector.scalar_tensor_tensor(
                            out=v_t, in0=q_t, scalar=2.0, in1=u_t,
                            op0=OP.mult, op1=OP.mult))
                        prev = (v_t, zh_t, cH_t, czh_t)
                        prev_hh = hh
                        # 14. output transpose piece (even steps)
                        if t % 2 == 0 and prev_out is not None:
                            oring, ot0, osc = prev_out
                            j = t // 2
                            yt = emit_out_transpose(oring, j, osc)
                            pend_yt = (yt, ot0, osc, j)
                        # 15. one GEMM piece
                        if pi < len(nxt):
                            nxt[pi]()
                            pi += 1
                    while pi < len(nxt):
                        nxt[pi]()
                        pi += 1
                    prev_out = (ring, t0, sc)
                    prev_ring = ring

                # tail: final state write + flush last chunk's output
                v_p, zh_p, _, _ = prev
                hh_last = prev_ring[:, (TC - 1) * FS:TC * FS]
                vec_dep(nc.vector.tensor_tensor(hh_last, v_p, zh_p, OP.add))
                oring, ot0, osc = prev_out
                for j in range(TC * FS // 128):
                    yt = emit_out_transpose(oring, j, osc)
                    emit_out_store(yt, ot0, j, osc)
            else:
                # general mz/mr path: straightforward scan (not perf-tuned)
                prev_state = h_init
                prev_out = None
                for sc in range(NCH):
                    ring = ring_pool.tile([128, TC * FS], fp32, tag="ring",
                                          name=f"ring_{sc}")
                    nxt = make_gemm_pieces(sc + 2) if sc + 2 < NCH else []
                    XZ, XR, XH = chunk_gates[sc]
                    t0 = sc * TC
                    pi = 0
                    for t in range(TC):
                        fs = slice(t * FS, (t + 1) * FS)
                        hh = prev_state

                        def stile(tag):
                            return scan_pool.tile([128, FS], fp32, tag=tag,
                                                  name=f"{tag}_{sc}_{t}")

                        m2 = stile("m2")
                        vec_dep(nc.vector.tensor_tensor(
                            m2, hh, mr2t, OP.mult))
                        e2 = stile("e2")
                        vec_dep(nc.vector.tensor_tensor(
                            e2, m2, XR[:, fs], OP.add))
                        m1 = stile("m1")
                        gps_dep(nc.gpsimd.tensor_tensor(
                            m1, hh, mzt, OP.mult))
                        e1 = stile("e1")
                        gps_dep(nc.gpsimd.tensor_tensor(
                            e1, m1, XZ[:, fs], OP.add))
                        s_t = stile("s")
                        act_dep(nc.scalar.activation(s_t, e2, AF.Sigmoid))
                        z_t = stile("z")
                        act_dep(nc.scalar.activation(z_t, e1, AF.Sigmoid))
                        p_t = stile("p")
                        vec_dep(nc.vector.scalar_tensor_tensor(
                            out=p_t, in0=hh, scalar=1.0, in1=s_t,
                            op0=OP.subtract, op1=OP.mult))
                        e3 = stile("e3")
                        vec_dep(nc.vector.scalar_tensor_tensor(
                            out=e3, in0=p_t, scalar=4.0, in1=XH[:, fs],
                            op0=OP.mult, op1=OP.add))
                        q_t = stile("q")
                        act_dep(nc.scalar.activation(q_t, e3, AF.Sigmoid))
                        u_t = stile("u")
                        vec_dep(nc.vector.tensor_scalar(
                            out=u_t, in0=z_t, scalar1=-1.0, scalar2=1.0,
                            op0=OP.mult, op1=OP.add))
                        zh_t = stile("zh")
                        gps_dep(nc.gpsimd.tensor_tensor(
                            zh_t, z_t, hh, OP.mult))
                        v_t = stile("v")
                        vec_dep(nc.vector.scalar_tensor_tensor(
                            out=v_t, in0=q_t, scalar=2.0, in1=u_t,
                            op0=OP.mult, op1=OP.mult))
                        vec_dep(nc.vector.tensor_tensor(
                            ring[:, fs], v_t, zh_t, OP.add))
                        prev_state = ring[:, fs]
                        if pi < len(nxt):
                            nxt[pi]()
                            pi += 1
                        if t % 2 == 1 and prev_out is not None:
                            oring, ot0, osc = prev_out
                            j = (t - 1) // 2
                            yt = emit_out_transpose(oring, j, osc)
                            emit_out_store(yt, ot0, j, osc)
                    while pi < len(nxt):
                        nxt[pi]()
                        pi += 1
                    prev_out = (ring, t0, sc)
                oring, ot0, osc = prev_out
                for j in range(TC * FS // 128):
                    yt = emit_out_transpose(oring, j, osc)
                    emit_out_store(yt, ot0, j, osc)

    _legalize_sync_waits(nc)
    return nc


def _get_nc(fast: bool):
    if fast not in _cache:
        _cache[fast] = _build(fast)
    return _cache[fast]


LAST_RESULT = None


def kernel(**inputs):
    global LAST_RESULT
    import ml_dtypes
    from concourse.bass_utils import run_bass_kernel_spmd

    bf16 = ml_dtypes.bfloat16

    x = np.ascontiguousarray(np.asarray(inputs["x"], dtype=np.float32))
    kz = np.asarray(inputs["kz"], dtype=np.float32)
    kr = np.asarray(inputs["kr"], dtype=np.float32)
    kh = np.asarray(inputs["kh"], dtype=np.float32)
    mz = np.asarray(inputs["mz"], dtype=np.float32)
    mr = np.asarray(inputs["mr"], dtype=np.float32)
    br = np.asarray(inputs["br"], dtype=np.float32)
    bz = np.asarray(inputs["bz"], dtype=np.float32)
    assert x.shape == (B, T, D) and kz.shape == (D, H)

    fast = bool(np.all(mz == 1.0) and np.all(mr == 1.0))
    nc = _get_nc(fast)

    # [H] -> [128, HB] with [h_a, h_b] = v[h_b*128 + h_a]
    def pvec(v):
        return np.ascontiguousarray(v.reshape(HB, 128).T)

    bzv = pvec(bz - mz)
    if fast:
        brv = pvec(br - mr)
    else:
        brv = pvec(2.0 * (br - mr))
    kzb = np.ascontiguousarray(kz.astype(bf16))
    krb = np.ascontiguousarray(kr.astype(bf16))
    khb = np.ascontiguousarray(kh.astype(bf16))
    base = {"kz": kzb, "kr": krb, "kh": khb, "bzv": bzv, "brv": brv}
    if not fast:
        # [128, (hb, b)] tiles of mz / 2*mr broadcast over b
        def ptile(v):
            t = v.reshape(HB, 128).T  # [128, HB]
            return np.ascontiguousarray(
                np.repeat(t[:, :, None], BL, axis=2).reshape(128, FS))
        base["mzt"] = ptile(mz)
        base["mr2t"] = ptile(2.0 * mr)

    # Host pre-transpose + bf16 cast of x:
    # xt[p, (c, k, s, b, t)] = x[b, c*32+s*16+t, k*128+p]
    xbf = x.astype(bf16)
    in_maps = []
    for i in range(NCORES):
        xc = xbf[i * BL:(i + 1) * BL]                # [8, 512, 512]
        xv = xc.reshape(BL, NCH, 2, 16, KT, 128)     # b, c, s, t, k, p
        xt = xv.transpose(5, 1, 4, 2, 0, 3)          # p, c, k, s, b, t
        xt = np.ascontiguousarray(xt).reshape(128, NCH * KT * TC * BL)
        in_maps.append(dict(base, xt=xt))

    trace = bool(int(os.environ.get("KERNEL_TRACE", "0")))
    res = run_bass_kernel_spmd(nc, in_maps, list(range(NCORES)), trace=trace)
    LAST_RESULT = res
    # device returns shifted state hh = h + 1; un-shift on host
    ys = np.concatenate(
        [res.results[i]["ys"] for i in range(NCORES)], axis=0)
    ys -= 1.0
    return ys
